# revision 73
# baseline (speedup 1.0000x reference)
"""EBT MQA attention block for Trainium2, sharded over 8 NeuronCores.

Problem: B=2, S=2048, HID=2048, H=16 query heads, 1 KV head (MQA), D=128.
  qkv = hidden @ w_qkv; RoPE(q, k); attn = softmax(q k^T / sqrt(D)) @ v;
  out = attn_reshaped @ w_o.

Sharding: core c = 4*b + g handles batch b and query heads [4g, 4g+4).
The single KV head is recomputed on every core. Each core produces a
partial output hidden[b] contribution (its 4 heads through w_o rows);
the host sums the 4 partials per batch (partials come back bf16).

Host-side prep (free, not on HW critical path): hidden[b] transposed to
xT [HID, S] so the contraction dim lands on SBUF partitions; sin table
pre-negated on the first half (sin_pm) so RoPE needs no on-chip negation;
w_qkv columns / w_o rows sliced per head group.

Everything on the PE is bf16 (x, w, Q, K, V, attn weights, ao, w_o) and
the RoPE tables + output are bf16 too: rel err 9.0e-3 vs the 2e-2 gate.
bf16 Q/K makes the scores matmul ~14% faster than fp32r (257->225ns per
512-wide matmul). Softmax skips max-subtraction: scores*scale are O(+-6).

Phase 2/3 engine balance (per core):
- exp() of all S*S*4 scores is ~133us on the Act engine; scores matmuls
  are paired into 2-bank PSUM tiles so each ACTIVATE covers 1024 columns.
- softmax denominators: bf16 DVE add-tree over the 16 key tiles, then a
  PE-transpose onto query partitions so the reciprocal runs on free-size
  4 (DVE reciprocal is ~6.5ns per free element regardless of partitions).
  Normalization is a per-partition tensor_scalar inside a transpose
  sandwich that restores the [d, q] layout the o-projection needs.
- the o-projection is interleaved at matmul-group granularity between
  scores pairs of the next query chunk (PE fills the Act-paced bubbles);
  qc0 has no such filler so its AV accumulation runs progressively, two
  key tiles behind the exp stream (qc0 is Act-bound: 8.3us exp vs 7.9us
  PE per chunk-head). Chunk-head (0,0)'s scores+exp are prefilled into
  expT slot 0 during phase 1's last s-chunk (the Act engine has ~17us of
  slack there; the singles reuse the tv PSUM ring, which is dead after
  V) so phase 2 opens with its add-tree/AV immediately. Prefilling MORE
  heads ((0,1) fully or kt0-7) regressed: the last phase-1 chunk becomes
  the bottleneck (+5.2-7.6us) - the slack fits only one head.
- endgame (last chunk-head): the final o-proj group plus dt0-2 partial
  accumulations of two trailing groups pad the PE while the DVE add-tree
  drains; the aoT flush runs per 128-q block so the trailing o-projection
  starts on block 0. Trailing qt 12-15 groups run in fresh pools with
  6-deep PSUM/SBUF rings, copies alternating Act/DVE.
- expT lives in a persistent 2-slot ring allocated BEFORE the phase-1
  pools: if it aliases the RoPE scratch, the first exp waits ~4.5us for
  the last RoPE chain to release the addresses. The Act exp table is
  warmed by a dummy activation at t=0.

Hard-won facts (do not relearn these the slow way):
- A NEFF containing ANY collective (AllGather K/V dedup was fully built
  and worked) runs EVERY PE matmul ~17% slower (229->273ns), costing
  +44us against the 20us of deduped projection work - and the down-clock
  PERSISTS on the device after the run until NEURON_RT_RESET_CORES=1.
- Engines block IN-ORDER on dma_start wait conditions: a DMA whose input
  isn't ready stalls every later enqueue on that engine's queues. The
  RoPE half-swap DMAs must stay on nc.sync AFTER that chunk's xt bulk
  (1-ahead prefetch); 2-ahead xt stacks ~14us of swap delay and starves
  the psT ring. TRN2 HWDGE engines: SP and Activation (separate 16-queue
  sets); gpsimd is SWDGE (cheap to idle, ~900ns per issue).
- Phase 1 is DMA-front-bound: ~12.4MB of input at ~300GB/s effective
  paces chunks 0-1; psT busy 87us of a ~91us phase-1 wall.

Measured 303867/305461ns (session start 322077, original baseline
376685). Trace at end: PE-array busy ~271us over a ~288us span; gaps:
qc0 Act-bound equilibrium ~3us, endgame ~4.6us, phase-1 front ~3us,
phase1->2 transition ~2.5us; ~11us pre (runtime preamble + first DMA)
and ~12us post (tile-context teardown: ~57 EVENT_SEMAPHORE drains per
sequencer) outside the PE span - both framework-fixed.
"""

import os

# A NEFF that ever contained collectives leaves the NeuronCores down-clocked
# (~17% on every PE matmul) until the cores are reset; reset defensively so
# a contaminated device doesn't distort this kernel's timing.
os.environ.setdefault("NEURON_RT_RESET_CORES", "1")

import ml_dtypes
import numpy as np

import concourse.bass as bass
import concourse.mybir as mybir
import concourse.tile as tile
from concourse import bacc
from concourse.bass_utils import run_bass_kernel_spmd
from concourse.masks import make_identity

P = 128
S = 2048
HID = 2048
H = 16
HPC = 4  # query heads per core
D = 128
SCALE = 1.0 / np.sqrt(D)
NST = S // P  # 16 sequence tiles
NHT = HID // P  # 16 hidden (contraction) tiles
NQC = 4  # q-chunks of 512
QCW = S // NQC
NHC = 4  # hid chunks of 512 for the o-projection
HCW = HID // NHC
NKP = NST // 2  # 8 pairs of key tiles
QCOLS = HPC * D  # 512 q columns per core
KVCOLS = 2 * D  # 256
WCOLS = QCOLS + KVCOLS  # 768
F32 = mybir.dt.float32
F32R = mybir.dt.float32r
BF16 = mybir.dt.bfloat16
MULT = mybir.AluOpType.mult
ADD = mybir.AluOpType.add
AF = mybir.ActivationFunctionType


def build_nc(phases=(1, 2, 3)):
    nc = bacc.Bacc("TRN2")

    # bf16 inputs halve the 23MB input DMA: the projection accumulates in
    # fp32 PSUM, and bf16 runs at the same 1 cycle/row as fp32r on the PE
    xT_d = nc.dram_tensor("xT", [HID, S], BF16, kind="ExternalInput").ap()
    wcat_d = nc.dram_tensor("wcat", [HID, WCOLS], BF16, kind="ExternalInput").ap()
    wo_d = nc.dram_tensor("wo", [QCOLS, HID], BF16, kind="ExternalInput").ap()
    # bf16 RoPE tables: halves 2MB of the BW-bound phase-1 input stream
    cosT_d = nc.dram_tensor("cosT", [D, S], BF16, kind="ExternalInput").ap()
    sinTpm_d = nc.dram_tensor("sinTpm", [D, S], BF16, kind="ExternalInput").ap()
    # bf16 output: halves the 16MB of out stores (the host accumulates the
    # 4 partials per batch in fp32 anyway, and output scale is ~0.25 so the
    # added quantization noise is ~0.2% against a 2% gate)
    out_d = nc.dram_tensor("out", [S, HID], BF16, kind="ExternalOutput").ap()

    with tile.TileContext(nc) as tc:
        with tc.tile_pool(name="pers", bufs=1) as pers:
            # ---- persistent SBUF state ----
            # bf16 Q/K: the scores matmul runs ~14% faster than fp32r (257->
            # 225ns per 512-wide matmul) and the PE stays in bf16 mode all of
            # phase 2. Logit noise from bf16 rounding is ~0.4% and washes out
            # in the softmax average (measured rel err 7.5e-3 vs 2e-2 gate).
            qT_sb = pers.tile([P, HPC, NST, P], BF16)  # Q^T per head [d, s]
            kT_sb = pers.tile([P, NST, P], BF16)  # K^T [d, s]
            # bf16: the walrus verifier forbids mixing 32-bit and 16-bit
            # matmul inputs, so everything the bf16 expT touches is bf16
            v_sb = pers.tile([P, NST, D], BF16)  # V natural [s, d]
            ident = pers.tile([P, P], F32)
            ident_bf = pers.tile([P, P], BF16)
            # exp-score tiles: allocated BEFORE the phase-1 pools so they never
            # alias the RoPE scratch - otherwise the first phase-2 exp waits
            # ~4.5us for the last chunk's RoPE chain to release the addresses.
            # Manual 2-deep ring via the leading index.
            expT2_sb = pers.tile([P, 2, NST, QCW], BF16)

            make_identity(nc, ident[:])
            nc.vector.tensor_copy(ident_bf[:], ident[:])
            # warm the Act exp table so ACT_TABLE_LOAD doesn't fire at the
            # first real score exp
            warm = pers.tile([P, 1], BF16)
            nc.scalar.activation(warm[:], ident[:, 0:1], AF.Exp, scale=float(SCALE))

            # ====== Phase 1: QKV^T projection + transposed-domain RoPE ======
            # out^T orientation: stationary = w tiles [hid, col], moving =
            # x^T [hid, s] in 512-wide s-chunks. Q^T / K^T come out directly
            # in the layout the scores matmul wants; only V needs PE
            # transposes (16 blocks). RoPE in [d, s] layout: the half-swap is
            # a partition swap done with two SBUF->SBUF DMA copies; the sign
            # lives in the host-prepped sinTpm table.
            if 1 not in phases:
                nc.vector.memset(qT_sb[:, 0, 0, 0:1], 0.0)
            if 1 in phases:
              with (
                tc.tile_pool(name="p1sb", bufs=2) as p1sb,
                tc.tile_pool(name="p1w", bufs=1) as p1w,
                tc.tile_pool(name="p1ps", bufs=2, space="PSUM") as p1ps,
            ):
                w_sb = p1w.tile([P, NHT, WCOLS], BF16)
                wcat_r = wcat_d.rearrange("(ht p) c -> p ht c", p=P)
                cosT_sb = p1w.tile([P, S], BF16)
                sinT_sb = p1w.tile([P, S], BF16)

                SCW = 512  # s-chunk width
                NSC = S // SCW

                def issue_xt_dma(sc):
                    xt = p1sb.tile(
                        [P, NHT, SCW], BF16, tag="xt", bufs=3, name="xt"
                    )
                    xr = xT_d[:, sc * SCW : (sc + 1) * SCW].rearrange(
                        "(ht p) s -> p ht s", p=P
                    )
                    for ht in range(NHT):
                        nc.sync.dma_start(xt[:, ht, :], xr[:, ht, :])
                    return xt

                # first chunk: interleave per-ht weight and xT slices so the
                # first matmuls unblock after ~0.7MB instead of 8.3MB (the
                # single HWDGE queue progresses its in-flight window in
                # parallel, so early bytes gate the PE start)
                xt_next = p1sb.tile(
                    [P, NHT, SCW], BF16, tag="xt", bufs=3, name="xt"
                )
                xT_r0 = xT_d[:, 0:SCW].rearrange("(ht p) s -> p ht s", p=P)
                for ht in range(NHT):
                    nc.sync.dma_start(w_sb[:, ht, :], wcat_r[:, ht, :])
                    nc.sync.dma_start(xt_next[:, ht, :], xT_r0[:, ht, :])
                nc.sync.dma_start(cosT_sb[:], cosT_d)
                nc.sync.dma_start(sinT_sb[:], sinTpm_d)

                NCT = WCOLS // P  # 6 col-tiles: 0-3 q heads, 4 k, 5 v
                pending_tv = [None]  # V transposes deferred one col-tile
                for sc in range(NSC):
                    xt = xt_next
                    if sc + 1 < NSC:
                        xt_next = issue_xt_dma(sc + 1)
                    ssl = slice(sc * SCW, (sc + 1) * SCW)
                    # last chunk: K and V first so their RoPE/transpose chains
                    # drain while the q heads run -> phase 2 starts sooner.
                    # (A K/V AllGather dedup across the batch group was tried
                    # and reverted: a NEFF containing collectives runs ALL PE
                    # matmuls ~17% slower - 229->273ns - costing +44us against
                    # the 20us of deduped projection work.)
                    ct_order = (
                        (4, 5, 0, 1, 2, 3) if sc == NSC - 1 else range(NCT)
                    )
                    for pos, ct in enumerate(ct_order):
                        psT = p1ps.tile(
                            [P, SCW], F32, tag="psT", bufs=6, name="psT"
                        )
                        for ht in range(NHT):
                            nc.tensor.matmul(
                                psT[:],
                                w_sb[:, ht, ct * P : (ct + 1) * P],
                                xt[:, ht, :],
                                start=(ht == 0),
                                stop=(ht == NHT - 1),
                            )
                        if pending_tv[0] is not None:
                            pending_tv[0]()
                            pending_tv[0] = None
                        if sc == NSC - 1 and pos >= 2:
                            # prefill chunk-head (qc0, h0)'s scores + exp in
                            # the Act engine's phase-1 slack: the exp of the
                            # first phase-2 chunk-head is off its critical
                            # path entirely (qc0 is Act-bound). Reuses the
                            # tv PSUM ring (dead after V at pos 1); emitted
                            # before this ct's RoPE so the exps aren't
                            # queued behind the Act raw copies.
                            rhs_q00 = qT_sb[:, 0, 0:4, :].rearrange(
                                "p a b -> p (a b)"
                            )
                            for kt in range(4 * (pos - 2), 4 * (pos - 2) + 4):
                                psE = p1ps.tile(
                                    [P, SCW], F32, tag="tv", bufs=2, name="psE"
                                )
                                nc.tensor.matmul(
                                    psE[:],
                                    kT_sb[:, kt, :],
                                    rhs_q00,
                                    start=True,
                                    stop=True,
                                )
                                nc.scalar.activation(
                                    expT2_sb[:, 0, kt, :],
                                    psE[:],
                                    AF.Exp,
                                    scale=float(SCALE),
                                )
                        if ct < HPC + 1:
                            # RoPE for q heads (ct<4) and k (ct==4)
                            raw = p1sb.tile([P, SCW], F32, tag="raw")
                            nc.scalar.copy(raw[:], psT[:])
                            rot = p1sb.tile([P, SCW], F32, tag="rot")
                            nc.sync.dma_start(rot[0 : P // 2, :], raw[P // 2 : P, :])
                            nc.sync.dma_start(rot[P // 2 : P, :], raw[0 : P // 2, :])
                            tmp = p1sb.tile([P, SCW], F32, tag="tmp")
                            nc.vector.tensor_tensor(
                                tmp[:], rot[:], sinT_sb[:, ssl], MULT
                            )
                            if ct < HPC:
                                dst = qT_sb[:, ct, 4 * sc : 4 * (sc + 1), :]
                            else:
                                dst = kT_sb[:, 4 * sc : 4 * (sc + 1), :]
                            dst = dst.rearrange("p a b -> p (a b)")
                            nc.vector.tensor_tensor(
                                dst, psT[:], cosT_sb[:, ssl], MULT
                            )
                            nc.vector.tensor_add(dst, dst, tmp[:])
                        else:
                            # V: transpose [d, s] -> natural [s, d] blocks.
                            # The PE transposes are deferred until after the
                            # NEXT col-tile's matmuls so the PE doesn't wait
                            # on the vTs Act copy.
                            vTs = p1sb.tile([P, SCW], F32, tag="vTs")
                            nc.scalar.copy(vTs[:], psT[:])

                            def make_tv(vTs=vTs, sc=sc):
                                def emit():
                                    tv = p1ps.tile(
                                        [P, SCW], F32, tag="tv", bufs=2,
                                        name="tv",
                                    )
                                    for j in range(4):
                                        nc.tensor.transpose(
                                            tv[:, j * P : (j + 1) * P],
                                            vTs[:, j * P : (j + 1) * P],
                                            ident[:],
                                        )
                                    nc.scalar.copy(
                                        v_sb[:, 4 * sc : 4 * (sc + 1), :],
                                        tv[:].rearrange(
                                            "p (a b) -> p a b", a=4
                                        ),
                                    )
                                return emit

                            pending_tv[0] = make_tv()
                if pending_tv[0] is not None:
                    pending_tv[0]()
                    pending_tv[0] = None

            # ====== Phase 2+3: attention with interleaved o-projection ======
            # o-proj weights: DMA'd at phase-2 start so the load overlaps
            # the first attention chunks (first use is one q-chunk later).
            wo_sb, wo_free = tc.tile([P, HPC, HID], BF16, name="wo_sb")
            aoT_sb, aoT_free = tc.tile([P, HPC, S], BF16, name="aoT_sb")
            nc.sync.dma_start(
                wo_sb[:], wo_d.rearrange("(dt p) c -> p dt c", p=P)
            )
            if 2 in phases:
              with (
                tc.tile_pool(name="p2sb", bufs=2) as p2sb,
                tc.tile_pool(name="p2ps", bufs=2, space="PSUM") as p2ps,
                tc.tile_pool(name="p3sb", bufs=3) as p3sb,
            ):

                def ph3_group(qt, hc, act_copy=False):
                    # o-projection for one [128 q x 512 hid] block; issued
                    # between scores pairs so the PE never idles on Act.
                    psP = p2ps.tile([P, HCW], F32, tag="psP", bufs=2, name="psP")
                    for dt in range(HPC):
                        nc.tensor.matmul(
                            psP[:],
                            aoT_sb[:, dt, qt * P : (qt + 1) * P],
                            wo_sb[:, dt, hc * HCW : (hc + 1) * HCW],
                            start=(dt == 0),
                            stop=(dt == HPC - 1),
                        )
                    outst = p3sb.tile([P, HCW], BF16, tag="outst", bufs=3)
                    # alternate the PSUM->SBUF move between Act and DVE so
                    # neither engine becomes the chunk pacer
                    if act_copy or hc % 2:
                        nc.scalar.copy(outst[:], psP[:])
                    else:
                        nc.vector.tensor_copy(outst[:], psP[:])
                    nc.sync.dma_start(
                        out_d[qt * P : (qt + 1) * P, hc * HCW : (hc + 1) * HCW],
                        outst[:],
                    )

                # shared PSUM scratch: [:,0] holds zpart^T blocks, [:,1] the
                # transposed normalized attention output (both bf16, 1 bank)
                zmix = p2ps.tile([P, 2, 4, D], BF16, tag="zmix", bufs=1)
                pending_aot = [None]  # deferred aoT transposes of prev chunk

                for qc in range(NQC):
                    for h in range(HPC):
                        qprev = 4 * (qc - 1) + h  # o-proj tile interleaved here
                        # last chunk-head: defer the o-proj interleave into the
                        # epilogue, where the PE otherwise idles on the DVE
                        # normalization chain
                        last_qh = qc == NQC - 1 and h == HPC - 1
                        expT = expT2_sb[:, (4 * qc + h) % 2]
                        rhs_q = qT_sb[:, h, 4 * qc : 4 * (qc + 1), :]
                        # qc 0 has no o-proj interleave to fill the PE while
                        # the Act engine drains exp (the psS2 ring stalls it
                        # ~1us per head): run the AV accumulation
                        # progressively, two key tiles behind the exp stream
                        psAO = (
                            p2ps.tile(
                                [P, QCW], F32, tag="psAO", bufs=1, name="psAO"
                            )
                            if qc == 0
                            else None
                        )
                        # (0,0)'s scores+exp were prefilled into expT slot 0
                        # during phase 1's last chunk - skip straight to the
                        # add-tree/AV
                        kp_range = () if qc == 0 and h == 0 else range(NKP)
                        for kp in kp_range:
                            # two scores matmuls into adjacent PSUM banks, one
                            # 1024-wide exp over both
                            psS2 = p2ps.tile(
                                [P, 2, QCW], F32, tag="psS2", bufs=2, name="psS2"
                            )
                            for j in range(2):
                                nc.tensor.matmul(
                                    psS2[:, j, :],
                                    kT_sb[:, 2 * kp + j, :],
                                    rhs_q,
                                    start=True,
                                    stop=True,
                                )
                            nc.scalar.activation(
                                expT[:, 2 * kp : 2 * kp + 2, :],
                                psS2[:],
                                AF.Exp,
                                scale=float(SCALE),
                            )
                            if kp == 0 and pending_aot[0] is not None:
                                # prev chunk's DVE chain is done by now; the
                                # transposes slot into the exp-paced bubbles
                                pending_aot[0]()
                                pending_aot[0] = None
                            if qc >= 1 and kp in (3, 5, 7):
                                ph3_group(qprev, (kp - 3) // 2)
                            if qc == 0 and kp >= 2:
                                for kt in (2 * (kp - 2), 2 * (kp - 2) + 1):
                                    nc.tensor.matmul(
                                        psAO[:],
                                        v_sb[:, kt, :],
                                        expT[:, kt, :],
                                        start=(kt == 0),
                                        stop=False,
                                    )
                        # bf16 DVE add-tree: sum the 16 key tiles per partition
                        if last_qh:
                            # unbalanced tree: everything except the last two
                            # key tiles sums while the exps still stream, so
                            # only ~1.3us of DVE chain remains after the last
                            # exp (the balanced tree leaves 3.4us, which the
                            # endgame cannot hide)
                            t1 = p2sb.tile([P, 4, QCW], BF16, tag="red4", bufs=1)
                            nc.vector.tensor_tensor(
                                t1[:], expT[:, 0:4, :], expT[:, 4:8, :], ADD
                            )
                            t2 = p2sb.tile([P, 2, QCW], BF16, tag="red2", bufs=1)
                            nc.vector.tensor_tensor(
                                t2[:], t1[:, 0:2, :], t1[:, 2:4, :], ADD
                            )
                            u = p2sb.tile([P, 2, QCW], BF16, tag="red2b", bufs=1)
                            nc.vector.tensor_tensor(
                                u[:], expT[:, 8:10, :], expT[:, 10:12, :], ADD
                            )
                            za = p2sb.tile([P, 2, QCW], BF16, tag="red2c", bufs=1)
                            nc.vector.tensor_tensor(
                                za[:, 0, :], t2[:, 0, :], t2[:, 1, :], ADD
                            )
                            nc.vector.tensor_tensor(
                                za[:, 1, :], u[:, 0, :], u[:, 1, :], ADD
                            )
                            s013 = p2sb.tile([P, 2, QCW], BF16, tag="red2d", bufs=1)
                            # sum(0-11) and sum(12,13): ready after exp pair 6
                            nc.vector.tensor_tensor(
                                s013[:, 0, :], za[:, 0, :], za[:, 1, :], ADD
                            )
                            nc.vector.tensor_tensor(
                                s013[:, 1, :], expT[:, 12, :], expT[:, 13, :], ADD
                            )
                            zpart = p2sb.tile([P, QCW], BF16, tag="zpart", bufs=2)
                            nc.vector.tensor_tensor(
                                zpart[:], s013[:, 0, :], s013[:, 1, :], ADD
                            )
                            nc.vector.tensor_add(zpart[:], zpart[:], expT[:, 14, :])
                            nc.vector.tensor_add(zpart[:], zpart[:], expT[:, 15, :])
                        else:
                            red8 = p2sb.tile([P, 8, QCW], BF16, tag="red8", bufs=1)
                            nc.vector.tensor_tensor(
                                red8[:], expT[:, 0:8, :], expT[:, 8:16, :], ADD
                            )
                            red4 = p2sb.tile([P, 4, QCW], BF16, tag="red4", bufs=1)
                            nc.vector.tensor_tensor(
                                red4[:], red8[:, 0:4, :], red8[:, 4:8, :], ADD
                            )
                            red2 = p2sb.tile([P, 2, QCW], BF16, tag="red2", bufs=1)
                            nc.vector.tensor_tensor(
                                red2[:], red4[:, 0:2, :], red4[:, 2:4, :], ADD
                            )
                            zpart = p2sb.tile([P, QCW], BF16, tag="zpart", bufs=2)
                            nc.vector.tensor_tensor(
                                zpart[:], red2[:, 0, :], red2[:, 1, :], ADD
                            )
                        # attention output with V stationary: 16 wide matmuls
                        # (the natural [q,d] orientation needs 64 128-row
                        # matmuls per chunk and measures ~2x ideal from
                        # per-instruction/ldweights overhead)
                        if qc == 0:
                            # h0: all 16 (prefilled exps, nothing to pace
                            # behind); h>0: the tail of the progressive run
                            for kt in range(0 if h == 0 else 12, NST):
                                nc.tensor.matmul(
                                    psAO[:],
                                    v_sb[:, kt, :],
                                    expT[:, kt, :],
                                    start=(kt == 0),
                                    stop=(kt == NST - 1),
                                )
                        else:
                            psAO = p2ps.tile([P, QCW], F32, tag="psAO", bufs=1)
                            for kt in range(NST):
                                nc.tensor.matmul(
                                    psAO[:],
                                    v_sb[:, kt, :],
                                    expT[:, kt, :],
                                    start=(kt == 0),
                                    stop=(kt == NST - 1),
                                )
                        part_ps = []
                        if last_qh:
                            # endgame PE filler while the DVE add-tree (which
                            # also gates on the last exp) drains: the final
                            # o-proj group of qc2, plus dt 0-2 partial
                            # accumulations of the first two trailing groups
                            # (they only need heads 0-2 of this chunk, all
                            # flushed); their dt=3 closes between the
                            # per-block flushes below
                            ph3_group(qprev, 3, act_copy=True)
                            for hcp in range(2):
                                psPp = p2ps.tile(
                                    [P, HCW], F32, tag="psP", bufs=2, name="psP"
                                )
                                for dt in range(HPC - 1):
                                    nc.tensor.matmul(
                                        psPp[:],
                                        aoT_sb[:, dt, 12 * P : 13 * P],
                                        wo_sb[:, dt, hcp * HCW : (hcp + 1) * HCW],
                                        start=(dt == 0),
                                        stop=False,
                                    )
                                part_ps.append(psPp)
                        # Z onto query partitions: transpose zpart blocks, then
                        # a free-dim reduce; reciprocal on free-size 4 is ~free
                        for j in range(4):
                            nc.tensor.transpose(
                                zmix[:, 0, j, :],
                                zpart[:, j * P : (j + 1) * P],
                                ident_bf[:],
                            )
                        if qc >= 1 and not last_qh:
                            ph3_group(qprev, 3)
                        # normalization sandwich: [d,q] PSUM -> bf16 -> [q,d]
                        # transposes -> per-partition 1/Z tensor_scalar ->
                        # transposes back (in the deferred flush)
                        aoU = p2sb.tile([P, QCW], BF16, tag="aoU", bufs=2)
                        nc.vector.tensor_copy(aoU[:], psAO[:])
                        for j in range(4):
                            nc.tensor.transpose(
                                zmix[:, 1, j, :],
                                aoU[:, j * P : (j + 1) * P],
                                ident_bf[:],
                            )
                        zq = p2sb.tile([P, 4], F32, tag="zq", bufs=1)
                        nc.vector.tensor_reduce(
                            zq[:], zmix[:, 0], mybir.AxisListType.X, ADD
                        )
                        zrT = p2sb.tile([P, 4], F32, tag="zrT", bufs=1)
                        nc.vector.reciprocal(zrT[:], zq[:])
                        ao_nat = p2sb.tile([P, 4, D], BF16, tag="ao_nat", bufs=2)
                        if last_qh:
                            # normalize/flush per 128-q block (the first
                            # trailing group only needs block 0); between
                            # blocks the PE closes the partial trailing
                            # groups (dt=3 + Act copy + store)
                            for j in range(4):
                                nc.vector.tensor_scalar_mul(
                                    ao_nat[:, j, :],
                                    zmix[:, 1, j, :],
                                    zrT[:, j : j + 1],
                                )
                                nc.tensor.transpose(
                                    zmix[:, 1, j, :],
                                    ao_nat[:, j, :],
                                    ident_bf[:],
                                )
                                # copy on Act: the DVE TS->copy chain would
                                # self-serialize and pace the flush blocks
                                nc.scalar.copy(
                                    aoT_sb[
                                        :, h, qc * QCW + j * P : qc * QCW + (j + 1) * P
                                    ],
                                    zmix[:, 1, j, :],
                                )
                                if j < len(part_ps):
                                    psPp = part_ps[j]
                                    nc.tensor.matmul(
                                        psPp[:],
                                        aoT_sb[:, 3, 12 * P : 13 * P],
                                        wo_sb[:, 3, j * HCW : (j + 1) * HCW],
                                        start=False,
                                        stop=True,
                                    )
                                    outst = p3sb.tile(
                                        [P, HCW], BF16, tag="outst", bufs=3
                                    )
                                    nc.scalar.copy(outst[:], psPp[:])
                                    nc.sync.dma_start(
                                        out_d[
                                            12 * P : 13 * P,
                                            j * HCW : (j + 1) * HCW,
                                        ],
                                        outst[:],
                                    )
                            continue
                        for qt in range(4):
                            nc.vector.tensor_scalar_mul(
                                ao_nat[:, qt, :],
                                zmix[:, 1, qt, :],
                                zrT[:, qt : qt + 1],
                            )

                        def make_flush(ao_nat=ao_nat, h=h, qc=qc):
                            def flush():
                                for j in range(4):
                                    nc.tensor.transpose(
                                        zmix[:, 1, j, :],
                                        ao_nat[:, j, :],
                                        ident_bf[:],
                                    )
                                nc.vector.tensor_copy(
                                    aoT_sb[:, h, qc * QCW : (qc + 1) * QCW],
                                    zmix[:, 1].rearrange("p a b -> p (a b)"),
                                )
                            return flush

                        pending_aot[0] = make_flush()

            # Trailing o-projection (qt 12-15): nothing left to hide behind,
            # so run it in fresh pools with wide PSUM/SBUF rings - the PE
            # streams matmul groups while Act/DVE alternate on the PSUM->SBUF
            # copies and the out DMAs drain.
            if 2 in phases:
              with (
                tc.tile_pool(name="pt_ps", bufs=2, space="PSUM") as pt_ps,
                tc.tile_pool(name="pt_sb", bufs=2) as pt_sb,
            ):
                flip = 0
                for qt in range(12, 16):
                    for hc in range(NHC):
                        if qt == 12 and hc < 2:
                            continue  # closed early in the endgame epilogue
                        psP = pt_ps.tile([P, HCW], F32, tag="psPt", bufs=6)
                        for dt in range(HPC):
                            nc.tensor.matmul(
                                psP[:],
                                aoT_sb[:, dt, qt * P : (qt + 1) * P],
                                wo_sb[:, dt, hc * HCW : (hc + 1) * HCW],
                                start=(dt == 0),
                                stop=(dt == HPC - 1),
                            )
                        outst = pt_sb.tile([P, HCW], BF16, tag="outst_t", bufs=6)
                        if flip % 2:
                            nc.scalar.copy(outst[:], psP[:])
                        else:
                            nc.vector.tensor_copy(outst[:], psP[:])
                        flip += 1
                        nc.sync.dma_start(
                            out_d[
                                qt * P : (qt + 1) * P,
                                hc * HCW : (hc + 1) * HCW,
                            ],
                            outst[:],
                        )

            aoT_free()
            wo_free()

    nc.compile()
    return nc


def _ensure_ntff_hook():
    """The container's antenv lacks axon_hooks; shim it and install the
    ctypes-based NTFF profile hook so trace=True works under axon."""
    try:
        from antenv.axon_hooks import get_axon_ntff_profile_hook  # noqa: F401

        return
    except ImportError:
        pass
    import sys
    import types

    mod = types.ModuleType("antenv.axon_hooks")
    mod._hook = None

    def set_axon_ntff_profile_hook(h):
        mod._hook = h

    def get_axon_ntff_profile_hook():
        return mod._hook

    mod.set_axon_ntff_profile_hook = set_axon_ntff_profile_hook
    mod.get_axon_ntff_profile_hook = get_axon_ntff_profile_hook
    sys.modules["antenv.axon_hooks"] = mod
    try:
        import antenv

        antenv.axon_hooks = mod
    except ImportError:
        pass
    try:
        from trn_agent_boot.trn_boot import _ntff_profile_via_ctypes

        set_axon_ntff_profile_hook(
            _ntff_profile_via_ctypes("/opt/axon/libaxon_pjrt.so")
        )
    except Exception:
        pass


_NC_CACHE = None


def _get_nc():
    global _NC_CACHE
    if _NC_CACHE is None:
        _NC_CACHE = build_nc()
    return _NC_CACHE


def kernel(hidden_states, cos, sin, w_qkv, w_o):
    hidden_states = np.asarray(hidden_states, dtype=np.float32)
    cos = np.asarray(cos, dtype=np.float32)
    sin = np.asarray(sin, dtype=np.float32)
    w_qkv = np.asarray(w_qkv, dtype=np.float32)
    w_o = np.asarray(w_o, dtype=np.float32)

    B = hidden_states.shape[0]
    assert hidden_states.shape == (B, S, HID)

    sin_pm = np.concatenate([-sin[:, : D // 2], sin[:, D // 2 :]], axis=1)
    sinTpm = np.ascontiguousarray(sin_pm.T.astype(ml_dtypes.bfloat16))
    cosT = np.ascontiguousarray(cos.T.astype(ml_dtypes.bfloat16))
    xT = [
        np.ascontiguousarray(hidden_states[b].T.astype(ml_dtypes.bfloat16))
        for b in range(B)
    ]
    wkv = w_qkv[:, H * D :]
    in_maps = []
    for b in range(B):
        for g in range(4):
            wcat = np.ascontiguousarray(
                np.concatenate(
                    [w_qkv[:, g * QCOLS : (g + 1) * QCOLS], wkv], axis=1
                ).astype(ml_dtypes.bfloat16)
            )
            wo_g = np.ascontiguousarray(
                w_o[g * QCOLS : (g + 1) * QCOLS, :].astype(
                    ml_dtypes.bfloat16
                )
            )
            in_maps.append(
                {
                    "xT": xT[b],
                    "wcat": wcat,
                    "wo": wo_g,
                    "cosT": cosT,
                    "sinTpm": sinTpm,
                }
            )

    nc = _get_nc()
    trace = bool(int(os.environ.get("EBT_TRACE", "0")))
    if trace:
        _ensure_ntff_hook()
    res = run_bass_kernel_spmd(
        nc, in_maps, core_ids=list(range(8)), trace=trace
    )
    if trace and res.exec_time_ns is not None:
        print(f"HW exec time: {res.exec_time_ns} ns")
        print(f"mean exec time: {res.mean_exec_time_ns} ns")
        if res.instructions_and_trace is not None:
            print(f"trace: {res.instructions_and_trace[1]}")

    parts = [np.asarray(r["out"], dtype=np.float32) for r in res.results]
    out = np.stack(
        [
            parts[0] + parts[1] + parts[2] + parts[3],
            parts[4] + parts[5] + parts[6] + parts[7],
        ],
        axis=0,
    )
    return out.astype(np.float32)



# revision 74
# speedup vs baseline: 1.0057x; 1.0057x over previous
"""EBT MQA attention block for Trainium2, sharded over 8 NeuronCores.

Problem: B=2, S=2048, HID=2048, H=16 query heads, 1 KV head (MQA), D=128.
  qkv = hidden @ w_qkv; RoPE(q, k); attn = softmax(q k^T / sqrt(D)) @ v;
  out = attn_reshaped @ w_o.

Sharding: core c = 4*b + g handles batch b and query heads [4g, 4g+4).
The single KV head is recomputed on every core. Each core produces a
partial output hidden[b] contribution (its 4 heads through w_o rows);
the host sums the 4 partials per batch (partials come back bf16).

Host-side prep (free, not on HW critical path): hidden[b] transposed to
xT [HID, S] so the contraction dim lands on SBUF partitions; sin table
pre-negated on the first half (sin_pm) so RoPE needs no on-chip negation;
w_qkv columns / w_o rows sliced per head group.

Everything on the PE is bf16 (x, w, Q, K, V, attn weights, ao, w_o) and
the RoPE tables + output are bf16 too: rel err 9.0e-3 vs the 2e-2 gate.
bf16 Q/K makes the scores matmul ~14% faster than fp32r (257->225ns per
512-wide matmul). Softmax skips max-subtraction: scores*scale are O(+-6).

Phase 2/3 engine balance (per core):
- exp() of all S*S*4 scores is ~133us on the Act engine; scores matmuls
  are paired into 2-bank PSUM tiles so each ACTIVATE covers 1024 columns.
- softmax denominators: bf16 DVE add-tree over the 16 key tiles, then a
  PE-transpose onto query partitions so the reciprocal runs on free-size
  4 (DVE reciprocal is ~6.5ns per free element regardless of partitions).
  Normalization is a per-partition tensor_scalar inside a transpose
  sandwich that restores the [d, q] layout the o-projection needs.
- the o-projection is interleaved at matmul-group granularity between
  scores pairs of the next query chunk (PE fills the Act-paced bubbles);
  qc0 has no such filler so its AV accumulation runs progressively, two
  key tiles behind the exp stream (qc0 is Act-bound: 8.3us exp vs 7.9us
  PE per chunk-head). Chunk-head (0,0)'s scores+exp are prefilled into
  expT slot 0 during phase 1's last s-chunk (the Act engine has ~17us of
  slack there; the singles reuse the tv PSUM ring, which is dead after
  V) so phase 2 opens with its add-tree/AV immediately. Prefilling MORE
  heads ((0,1) fully or kt0-7) regressed: the last phase-1 chunk becomes
  the bottleneck (+5.2-7.6us) - the slack fits only one head.
- endgame (last chunk-head): the final o-proj group plus dt0-2 partial
  accumulations of two trailing groups pad the PE while the DVE add-tree
  drains; the aoT flush runs per 128-q block so the trailing o-projection
  starts on block 0. Trailing qt 12-15 groups run in fresh pools with
  6-deep PSUM/SBUF rings, copies alternating Act/DVE.
- expT lives in a persistent 2-slot ring allocated BEFORE the phase-1
  pools: if it aliases the RoPE scratch, the first exp waits ~4.5us for
  the last RoPE chain to release the addresses. The Act exp table is
  warmed by a dummy activation at t=0.

Hard-won facts (do not relearn these the slow way):
- A NEFF containing ANY collective (AllGather K/V dedup was fully built
  and worked) runs EVERY PE matmul ~17% slower (229->273ns), costing
  +44us against the 20us of deduped projection work - and the down-clock
  PERSISTS on the device after the run until NEURON_RT_RESET_CORES=1.
- Engines block IN-ORDER on dma_start wait conditions: a DMA whose input
  isn't ready stalls every later enqueue on that engine's queues. The
  RoPE half-swap DMAs must stay on nc.sync AFTER that chunk's xt bulk
  (1-ahead prefetch); 2-ahead xt stacks ~14us of swap delay and starves
  the psT ring. TRN2 HWDGE engines: SP and Activation (separate 16-queue
  sets); gpsimd is SWDGE (cheap to idle, ~900ns per issue).
- Phase 1 is DMA-front-bound: ~12.4MB of input at ~300GB/s effective
  paces chunks 0-1; psT busy 87us of a ~91us phase-1 wall.

Measured 303867/305461ns (session start 322077, original baseline
376685). Trace at end: PE-array busy ~271us over a ~288us span; gaps:
qc0 Act-bound equilibrium ~3us, endgame ~4.6us, phase-1 front ~3us,
phase1->2 transition ~2.5us; ~11us pre (runtime preamble + first DMA)
and ~12us post (tile-context teardown: ~57 EVENT_SEMAPHORE drains per
sequencer) outside the PE span - both framework-fixed.
"""

import os

# A NEFF that ever contained collectives leaves the NeuronCores down-clocked
# (~17% on every PE matmul) until the cores are reset; reset defensively so
# a contaminated device doesn't distort this kernel's timing.
os.environ.setdefault("NEURON_RT_RESET_CORES", "1")

import ml_dtypes
import numpy as np

import concourse.bass as bass
import concourse.mybir as mybir
import concourse.tile as tile
from concourse import bacc
from concourse.bass_utils import run_bass_kernel_spmd
from concourse.masks import make_identity

P = 128
S = 2048
HID = 2048
H = 16
HPC = 4  # query heads per core
D = 128
SCALE = 1.0 / np.sqrt(D)
NST = S // P  # 16 sequence tiles
NHT = HID // P  # 16 hidden (contraction) tiles
NQC = 4  # q-chunks of 512
QCW = S // NQC
NHC = 4  # hid chunks of 512 for the o-projection
HCW = HID // NHC
NKP = NST // 2  # 8 pairs of key tiles
QCOLS = HPC * D  # 512 q columns per core
KVCOLS = 2 * D  # 256
WCOLS = QCOLS + KVCOLS  # 768
F32 = mybir.dt.float32
F32R = mybir.dt.float32r
BF16 = mybir.dt.bfloat16
MULT = mybir.AluOpType.mult
ADD = mybir.AluOpType.add
AF = mybir.ActivationFunctionType


def build_nc(phases=(1, 2, 3)):
    nc = bacc.Bacc("TRN2")

    # bf16 inputs halve the 23MB input DMA: the projection accumulates in
    # fp32 PSUM, and bf16 runs at the same 1 cycle/row as fp32r on the PE
    xT_d = nc.dram_tensor("xT", [HID, S], BF16, kind="ExternalInput").ap()
    wcat_d = nc.dram_tensor("wcat", [HID, WCOLS], BF16, kind="ExternalInput").ap()
    wo_d = nc.dram_tensor("wo", [QCOLS, HID], BF16, kind="ExternalInput").ap()
    # bf16 RoPE tables: halves 2MB of the BW-bound phase-1 input stream
    cosT_d = nc.dram_tensor("cosT", [D, S], BF16, kind="ExternalInput").ap()
    sinTpm_d = nc.dram_tensor("sinTpm", [D, S], BF16, kind="ExternalInput").ap()
    # bf16 output: halves the 16MB of out stores (the host accumulates the
    # 4 partials per batch in fp32 anyway, and output scale is ~0.25 so the
    # added quantization noise is ~0.2% against a 2% gate)
    out_d = nc.dram_tensor("out", [S, HID], BF16, kind="ExternalOutput").ap()

    with tile.TileContext(nc) as tc:
        with tc.tile_pool(name="pers", bufs=1) as pers:
            # ---- persistent SBUF state ----
            # bf16 Q/K: the scores matmul runs ~14% faster than fp32r (257->
            # 225ns per 512-wide matmul) and the PE stays in bf16 mode all of
            # phase 2. Logit noise from bf16 rounding is ~0.4% and washes out
            # in the softmax average (measured rel err 7.5e-3 vs 2e-2 gate).
            qT_sb = pers.tile([P, HPC, NST, P], BF16)  # Q^T per head [d, s]
            kT_sb = pers.tile([P, NST, P], BF16)  # K^T [d, s]
            # bf16: the walrus verifier forbids mixing 32-bit and 16-bit
            # matmul inputs, so everything the bf16 expT touches is bf16
            v_sb = pers.tile([P, NST, D], BF16)  # V natural [s, d]
            ident = pers.tile([P, P], F32)
            ident_bf = pers.tile([P, P], BF16)
            # exp-score tiles: allocated BEFORE the phase-1 pools so they never
            # alias the RoPE scratch - otherwise the first phase-2 exp waits
            # ~4.5us for the last chunk's RoPE chain to release the addresses.
            # Manual 2-deep ring via the leading index.
            expT2_sb = pers.tile([P, 2, NST, QCW], BF16)

            make_identity(nc, ident[:])
            nc.vector.tensor_copy(ident_bf[:], ident[:])
            # warm the Act exp table so ACT_TABLE_LOAD doesn't fire at the
            # first real score exp
            warm = pers.tile([P, 1], BF16)
            nc.scalar.activation(warm[:], ident[:, 0:1], AF.Exp, scale=float(SCALE))

            # ====== Phase 1: QKV^T projection + transposed-domain RoPE ======
            # out^T orientation: stationary = w tiles [hid, col], moving =
            # x^T [hid, s] in 512-wide s-chunks. Q^T / K^T come out directly
            # in the layout the scores matmul wants; only V needs PE
            # transposes (16 blocks). RoPE in [d, s] layout: the half-swap is
            # a partition swap done with two SBUF->SBUF DMA copies; the sign
            # lives in the host-prepped sinTpm table.
            if 1 not in phases:
                nc.vector.memset(qT_sb[:, 0, 0, 0:1], 0.0)
            if 1 in phases:
              with (
                tc.tile_pool(name="p1sb", bufs=2) as p1sb,
                tc.tile_pool(name="p1w", bufs=1) as p1w,
                tc.tile_pool(name="p1ps", bufs=2, space="PSUM") as p1ps,
            ):
                w_sb = p1w.tile([P, NHT, WCOLS], BF16)
                wcat_r = wcat_d.rearrange("(ht p) c -> p ht c", p=P)
                cosT_sb = p1w.tile([P, S], BF16)
                sinT_sb = p1w.tile([P, S], BF16)

                SCW = 512  # s-chunk width
                NSC = S // SCW

                def issue_xt_dma(sc):
                    xt = p1sb.tile(
                        [P, NHT, SCW], BF16, tag="xt", bufs=3, name="xt"
                    )
                    xr = xT_d[:, sc * SCW : (sc + 1) * SCW].rearrange(
                        "(ht p) s -> p ht s", p=P
                    )
                    for ht in range(NHT):
                        nc.sync.dma_start(xt[:, ht, :], xr[:, ht, :])
                    return xt

                # first chunk: interleave per-ht weight and xT slices so the
                # first matmuls unblock after ~0.7MB instead of 8.3MB (the
                # single HWDGE queue progresses its in-flight window in
                # parallel, so early bytes gate the PE start)
                xt_next = p1sb.tile(
                    [P, NHT, SCW], BF16, tag="xt", bufs=3, name="xt"
                )
                xT_r0 = xT_d[:, 0:SCW].rearrange("(ht p) s -> p ht s", p=P)
                for ht in range(NHT):
                    nc.sync.dma_start(w_sb[:, ht, :], wcat_r[:, ht, :])
                    nc.sync.dma_start(xt_next[:, ht, :], xT_r0[:, ht, :])
                nc.sync.dma_start(cosT_sb[:], cosT_d)
                nc.sync.dma_start(sinT_sb[:], sinTpm_d)

                NCT = WCOLS // P  # 6 col-tiles: 0-3 q heads, 4 k, 5 v
                pending_tv = [None]  # V transposes deferred one col-tile
                for sc in range(NSC):
                    xt = xt_next
                    if sc + 1 < NSC:
                        xt_next = issue_xt_dma(sc + 1)
                    ssl = slice(sc * SCW, (sc + 1) * SCW)
                    # last chunk: K and V first so their RoPE/transpose chains
                    # drain while the q heads run -> phase 2 starts sooner.
                    # (A K/V AllGather dedup across the batch group was tried
                    # and reverted: a NEFF containing collectives runs ALL PE
                    # matmuls ~17% slower - 229->273ns - costing +44us against
                    # the 20us of deduped projection work.)
                    ct_order = (
                        (4, 5, 0, 1, 2, 3) if sc == NSC - 1 else range(NCT)
                    )
                    for pos, ct in enumerate(ct_order):
                        psT = p1ps.tile(
                            [P, SCW], F32, tag="psT", bufs=6, name="psT"
                        )
                        for ht in range(NHT):
                            nc.tensor.matmul(
                                psT[:],
                                w_sb[:, ht, ct * P : (ct + 1) * P],
                                xt[:, ht, :],
                                start=(ht == 0),
                                stop=(ht == NHT - 1),
                            )
                        if pending_tv[0] is not None:
                            pending_tv[0]()
                            pending_tv[0] = None
                        if sc == NSC - 1 and pos >= 2:
                            # prefill chunk-head (qc0, h0)'s scores + exp in
                            # the Act engine's phase-1 slack: the exp of the
                            # first phase-2 chunk-head is off its critical
                            # path entirely (qc0 is Act-bound). Reuses the
                            # tv PSUM ring (dead after V at pos 1); emitted
                            # before this ct's RoPE so the exps aren't
                            # queued behind the Act raw copies.
                            rhs_q00 = qT_sb[:, 0, 0:4, :].rearrange(
                                "p a b -> p (a b)"
                            )
                            for kt in range(4 * (pos - 2), 4 * (pos - 2) + 4):
                                psE = p1ps.tile(
                                    [P, SCW], F32, tag="tv", bufs=2, name="psE"
                                )
                                nc.tensor.matmul(
                                    psE[:],
                                    kT_sb[:, kt, :],
                                    rhs_q00,
                                    start=True,
                                    stop=True,
                                )
                                nc.scalar.activation(
                                    expT2_sb[:, 0, kt, :],
                                    psE[:],
                                    AF.Exp,
                                    scale=float(SCALE),
                                )
                        if ct < HPC + 1:
                            # RoPE for q heads (ct<4) and k (ct==4)
                            raw = p1sb.tile([P, SCW], F32, tag="raw")
                            nc.scalar.copy(raw[:], psT[:])
                            rot = p1sb.tile([P, SCW], F32, tag="rot")
                            nc.sync.dma_start(rot[0 : P // 2, :], raw[P // 2 : P, :])
                            nc.sync.dma_start(rot[P // 2 : P, :], raw[0 : P // 2, :])
                            tmp = p1sb.tile([P, SCW], F32, tag="tmp")
                            nc.vector.tensor_tensor(
                                tmp[:], rot[:], sinT_sb[:, ssl], MULT
                            )
                            if ct < HPC:
                                dst = qT_sb[:, ct, 4 * sc : 4 * (sc + 1), :]
                            else:
                                dst = kT_sb[:, 4 * sc : 4 * (sc + 1), :]
                            dst = dst.rearrange("p a b -> p (a b)")
                            nc.vector.tensor_tensor(
                                dst, psT[:], cosT_sb[:, ssl], MULT
                            )
                            nc.vector.tensor_add(dst, dst, tmp[:])
                        else:
                            # V: transpose [d, s] -> natural [s, d] blocks.
                            # The PE transposes are deferred until after the
                            # NEXT col-tile's matmuls so the PE doesn't wait
                            # on the vTs Act copy.
                            vTs = p1sb.tile([P, SCW], F32, tag="vTs")
                            nc.scalar.copy(vTs[:], psT[:])

                            def make_tv(vTs=vTs, sc=sc):
                                def emit():
                                    tv = p1ps.tile(
                                        [P, SCW], F32, tag="tv", bufs=2,
                                        name="tv",
                                    )
                                    for j in range(4):
                                        nc.tensor.transpose(
                                            tv[:, j * P : (j + 1) * P],
                                            vTs[:, j * P : (j + 1) * P],
                                            ident[:],
                                        )
                                    nc.scalar.copy(
                                        v_sb[:, 4 * sc : 4 * (sc + 1), :],
                                        tv[:].rearrange(
                                            "p (a b) -> p a b", a=4
                                        ),
                                    )
                                return emit

                            pending_tv[0] = make_tv()
                if pending_tv[0] is not None:
                    pending_tv[0]()
                    pending_tv[0] = None

            # ====== Phase 2+3: attention with interleaved o-projection ======
            # o-proj weights: DMA'd at phase-2 start so the load overlaps
            # the first attention chunks (first use is one q-chunk later).
            wo_sb, wo_free = tc.tile([P, HPC, HID], BF16, name="wo_sb")
            aoT_sb, aoT_free = tc.tile([P, HPC, S], BF16, name="aoT_sb")
            nc.sync.dma_start(
                wo_sb[:], wo_d.rearrange("(dt p) c -> p dt c", p=P)
            )
            if 2 in phases:
              with (
                tc.tile_pool(name="p2sb", bufs=2) as p2sb,
                tc.tile_pool(name="p2ps", bufs=2, space="PSUM") as p2ps,
                tc.tile_pool(name="p3sb", bufs=3) as p3sb,
            ):

                def ph3_group(qt, hc, act_copy=False):
                    # o-projection for one [128 q x 512 hid] block; issued
                    # between scores pairs so the PE never idles on Act.
                    psP = p2ps.tile([P, HCW], F32, tag="psP", bufs=2, name="psP")
                    for dt in range(HPC):
                        nc.tensor.matmul(
                            psP[:],
                            aoT_sb[:, dt, qt * P : (qt + 1) * P],
                            wo_sb[:, dt, hc * HCW : (hc + 1) * HCW],
                            start=(dt == 0),
                            stop=(dt == HPC - 1),
                        )
                    outst = p3sb.tile([P, HCW], BF16, tag="outst", bufs=3)
                    # alternate the PSUM->SBUF move between Act and DVE so
                    # neither engine becomes the chunk pacer
                    if act_copy or hc % 2:
                        nc.scalar.copy(outst[:], psP[:])
                    else:
                        nc.vector.tensor_copy(outst[:], psP[:])
                    nc.sync.dma_start(
                        out_d[qt * P : (qt + 1) * P, hc * HCW : (hc + 1) * HCW],
                        outst[:],
                    )

                # shared PSUM scratch: [:,0] holds zpart^T blocks, [:,1] the
                # transposed normalized attention output (both bf16, 1 bank)
                zmix = p2ps.tile([P, 2, 4, D], BF16, tag="zmix", bufs=1)
                pending_aot = [None]  # deferred aoT transposes of prev chunk

                for qc in range(NQC):
                    for h in range(HPC):
                        qprev = 4 * (qc - 1) + h  # o-proj tile interleaved here
                        # last chunk-head: defer the o-proj interleave into the
                        # epilogue, where the PE otherwise idles on the DVE
                        # normalization chain
                        last_qh = qc == NQC - 1 and h == HPC - 1
                        expT = expT2_sb[:, (4 * qc + h) % 2]
                        rhs_q = qT_sb[:, h, 4 * qc : 4 * (qc + 1), :]
                        # qc 0 has no o-proj interleave to fill the PE while
                        # the Act engine drains exp (the psS2 ring stalls it
                        # ~1us per head): run the AV accumulation
                        # progressively, two key tiles behind the exp stream
                        psAO = (
                            p2ps.tile(
                                [P, QCW], F32, tag="psAO", bufs=1, name="psAO"
                            )
                            if qc == 0
                            else None
                        )
                        # (0,0)'s scores+exp were prefilled into expT slot 0
                        # during phase 1's last chunk - skip straight to the
                        # add-tree/AV
                        kp_range = () if qc == 0 and h == 0 else range(NKP)
                        for kp in kp_range:
                            # two scores matmuls into adjacent PSUM banks, one
                            # 1024-wide exp over both
                            psS2 = p2ps.tile(
                                [P, 2, QCW], F32, tag="psS2", bufs=2, name="psS2"
                            )
                            for j in range(2):
                                nc.tensor.matmul(
                                    psS2[:, j, :],
                                    kT_sb[:, 2 * kp + j, :],
                                    rhs_q,
                                    start=True,
                                    stop=True,
                                )
                            nc.scalar.activation(
                                expT[:, 2 * kp : 2 * kp + 2, :],
                                psS2[:],
                                AF.Exp,
                                scale=float(SCALE),
                            )
                            if kp == 0 and pending_aot[0] is not None:
                                # prev chunk's DVE chain is done by now; the
                                # transposes slot into the exp-paced bubbles
                                pending_aot[0]()
                                pending_aot[0] = None
                            if qc >= 1 and kp in (3, 5, 7):
                                ph3_group(qprev, (kp - 3) // 2)
                            if qc == 0 and kp >= 2:
                                for kt in (2 * (kp - 2), 2 * (kp - 2) + 1):
                                    nc.tensor.matmul(
                                        psAO[:],
                                        v_sb[:, kt, :],
                                        expT[:, kt, :],
                                        start=(kt == 0),
                                        stop=False,
                                    )
                        # bf16 DVE add-tree: sum the 16 key tiles per partition
                        if last_qh:
                            # unbalanced tree: everything except the last two
                            # key tiles sums while the exps still stream, so
                            # only ~1.3us of DVE chain remains after the last
                            # exp (the balanced tree leaves 3.4us, which the
                            # endgame cannot hide)
                            t1 = p2sb.tile([P, 4, QCW], BF16, tag="red4", bufs=1)
                            nc.vector.tensor_tensor(
                                t1[:], expT[:, 0:4, :], expT[:, 4:8, :], ADD
                            )
                            t2 = p2sb.tile([P, 2, QCW], BF16, tag="red2", bufs=1)
                            nc.vector.tensor_tensor(
                                t2[:], t1[:, 0:2, :], t1[:, 2:4, :], ADD
                            )
                            u = p2sb.tile([P, 2, QCW], BF16, tag="red2b", bufs=1)
                            nc.vector.tensor_tensor(
                                u[:], expT[:, 8:10, :], expT[:, 10:12, :], ADD
                            )
                            za = p2sb.tile([P, 2, QCW], BF16, tag="red2c", bufs=1)
                            nc.vector.tensor_tensor(
                                za[:, 0, :], t2[:, 0, :], t2[:, 1, :], ADD
                            )
                            nc.vector.tensor_tensor(
                                za[:, 1, :], u[:, 0, :], u[:, 1, :], ADD
                            )
                            s013 = p2sb.tile([P, 2, QCW], BF16, tag="red2d", bufs=1)
                            # sum(0-11) and sum(12,13): ready after exp pair 6
                            nc.vector.tensor_tensor(
                                s013[:, 0, :], za[:, 0, :], za[:, 1, :], ADD
                            )
                            nc.vector.tensor_tensor(
                                s013[:, 1, :], expT[:, 12, :], expT[:, 13, :], ADD
                            )
                            zpart = p2sb.tile([P, QCW], BF16, tag="zpart", bufs=2)
                            nc.vector.tensor_tensor(
                                zpart[:], s013[:, 0, :], s013[:, 1, :], ADD
                            )
                            nc.vector.tensor_add(zpart[:], zpart[:], expT[:, 14, :])
                            nc.vector.tensor_add(zpart[:], zpart[:], expT[:, 15, :])
                        else:
                            red8 = p2sb.tile([P, 8, QCW], BF16, tag="red8", bufs=1)
                            nc.vector.tensor_tensor(
                                red8[:], expT[:, 0:8, :], expT[:, 8:16, :], ADD
                            )
                            red4 = p2sb.tile([P, 4, QCW], BF16, tag="red4", bufs=1)
                            nc.vector.tensor_tensor(
                                red4[:], red8[:, 0:4, :], red8[:, 4:8, :], ADD
                            )
                            red2 = p2sb.tile([P, 2, QCW], BF16, tag="red2", bufs=1)
                            nc.vector.tensor_tensor(
                                red2[:], red4[:, 0:2, :], red4[:, 2:4, :], ADD
                            )
                            zpart = p2sb.tile([P, QCW], BF16, tag="zpart", bufs=2)
                            nc.vector.tensor_tensor(
                                zpart[:], red2[:, 0, :], red2[:, 1, :], ADD
                            )
                        # attention output with V stationary: 16 wide matmuls
                        # (the natural [q,d] orientation needs 64 128-row
                        # matmuls per chunk and measures ~2x ideal from
                        # per-instruction/ldweights overhead)
                        if qc == 0:
                            # h0: all 16 (prefilled exps, nothing to pace
                            # behind); h>0: the tail of the progressive run
                            for kt in range(0 if h == 0 else 12, NST):
                                nc.tensor.matmul(
                                    psAO[:],
                                    v_sb[:, kt, :],
                                    expT[:, kt, :],
                                    start=(kt == 0),
                                    stop=(kt == NST - 1),
                                )
                        else:
                            psAO = p2ps.tile([P, QCW], F32, tag="psAO", bufs=1)
                            for kt in range(NST):
                                nc.tensor.matmul(
                                    psAO[:],
                                    v_sb[:, kt, :],
                                    expT[:, kt, :],
                                    start=(kt == 0),
                                    stop=(kt == NST - 1),
                                )
                        part_ps = []
                        if last_qh:
                            # endgame PE filler while the DVE add-tree (which
                            # also gates on the last exp) drains: the final
                            # o-proj group of qc2, plus dt 0-2 partial
                            # accumulations of the first two trailing groups
                            # (they only need heads 0-2 of this chunk, all
                            # flushed); their dt=3 closes between the
                            # per-block flushes below
                            ph3_group(qprev, 3, act_copy=True)
                            for hcp in range(2):
                                psPp = p2ps.tile(
                                    [P, HCW], F32, tag="psP", bufs=2, name="psP"
                                )
                                for dt in range(HPC - 1):
                                    nc.tensor.matmul(
                                        psPp[:],
                                        aoT_sb[:, dt, 12 * P : 13 * P],
                                        wo_sb[:, dt, hcp * HCW : (hcp + 1) * HCW],
                                        start=(dt == 0),
                                        stop=False,
                                    )
                                part_ps.append(psPp)
                        # Z onto query partitions: transpose zpart blocks, then
                        # a free-dim reduce; reciprocal on free-size 4 is ~free
                        for j in range(4):
                            nc.tensor.transpose(
                                zmix[:, 0, j, :],
                                zpart[:, j * P : (j + 1) * P],
                                ident_bf[:],
                            )
                        if qc >= 1 and not last_qh:
                            ph3_group(qprev, 3)
                        # normalization sandwich: [d,q] PSUM -> bf16 -> [q,d]
                        # transposes -> per-partition 1/Z tensor_scalar ->
                        # transposes back (in the deferred flush)
                        aoU = p2sb.tile([P, QCW], BF16, tag="aoU", bufs=2)
                        nc.vector.tensor_copy(aoU[:], psAO[:])
                        for j in range(4):
                            nc.tensor.transpose(
                                zmix[:, 1, j, :],
                                aoU[:, j * P : (j + 1) * P],
                                ident_bf[:],
                            )
                        zq = p2sb.tile([P, 4], F32, tag="zq", bufs=1)
                        nc.vector.tensor_reduce(
                            zq[:], zmix[:, 0], mybir.AxisListType.X, ADD
                        )
                        zrT = p2sb.tile([P, 4], F32, tag="zrT", bufs=1)
                        nc.vector.reciprocal(zrT[:], zq[:])
                        ao_nat = p2sb.tile([P, 4, D], BF16, tag="ao_nat", bufs=2)
                        if last_qh:
                            # normalize/flush per 128-q block (the first
                            # trailing group only needs block 0); between
                            # blocks the PE closes the partial trailing
                            # groups (dt=3 + Act copy + store)
                            for j in range(4):
                                nc.vector.tensor_scalar_mul(
                                    ao_nat[:, j, :],
                                    zmix[:, 1, j, :],
                                    zrT[:, j : j + 1],
                                )
                                nc.tensor.transpose(
                                    zmix[:, 1, j, :],
                                    ao_nat[:, j, :],
                                    ident_bf[:],
                                )
                                # copy on Act: the DVE TS->copy chain would
                                # self-serialize and pace the flush blocks
                                nc.scalar.copy(
                                    aoT_sb[
                                        :, h, qc * QCW + j * P : qc * QCW + (j + 1) * P
                                    ],
                                    zmix[:, 1, j, :],
                                )
                                if j < len(part_ps):
                                    psPp = part_ps[j]
                                    nc.tensor.matmul(
                                        psPp[:],
                                        aoT_sb[:, 3, 12 * P : 13 * P],
                                        wo_sb[:, 3, j * HCW : (j + 1) * HCW],
                                        start=False,
                                        stop=True,
                                    )
                                    outst = p3sb.tile(
                                        [P, HCW], BF16, tag="outst", bufs=3
                                    )
                                    # DVE: Act already carries the per-block
                                    # flush copies
                                    nc.vector.tensor_copy(outst[:], psPp[:])
                                    nc.sync.dma_start(
                                        out_d[
                                            12 * P : 13 * P,
                                            j * HCW : (j + 1) * HCW,
                                        ],
                                        outst[:],
                                    )
                            continue
                        for qt in range(4):
                            nc.vector.tensor_scalar_mul(
                                ao_nat[:, qt, :],
                                zmix[:, 1, qt, :],
                                zrT[:, qt : qt + 1],
                            )

                        def make_flush(ao_nat=ao_nat, h=h, qc=qc):
                            def flush():
                                for j in range(4):
                                    nc.tensor.transpose(
                                        zmix[:, 1, j, :],
                                        ao_nat[:, j, :],
                                        ident_bf[:],
                                    )
                                nc.vector.tensor_copy(
                                    aoT_sb[:, h, qc * QCW : (qc + 1) * QCW],
                                    zmix[:, 1].rearrange("p a b -> p (a b)"),
                                )
                            return flush

                        pending_aot[0] = make_flush()

            # Trailing o-projection (qt 12-15): nothing left to hide behind,
            # so run it in fresh pools with wide PSUM/SBUF rings - the PE
            # streams matmul groups while Act/DVE alternate on the PSUM->SBUF
            # copies and the out DMAs drain.
            if 2 in phases:
              with (
                tc.tile_pool(name="pt_ps", bufs=2, space="PSUM") as pt_ps,
                tc.tile_pool(name="pt_sb", bufs=2) as pt_sb,
            ):
                flip = 0
                for qt in range(12, 16):
                    for hc in range(NHC):
                        if qt == 12 and hc < 2:
                            continue  # closed early in the endgame epilogue
                        psP = pt_ps.tile([P, HCW], F32, tag="psPt", bufs=6)
                        for dt in range(HPC):
                            nc.tensor.matmul(
                                psP[:],
                                aoT_sb[:, dt, qt * P : (qt + 1) * P],
                                wo_sb[:, dt, hc * HCW : (hc + 1) * HCW],
                                start=(dt == 0),
                                stop=(dt == HPC - 1),
                            )
                        outst = pt_sb.tile([P, HCW], BF16, tag="outst_t", bufs=6)
                        if flip % 2:
                            nc.scalar.copy(outst[:], psP[:])
                        else:
                            nc.vector.tensor_copy(outst[:], psP[:])
                        flip += 1
                        nc.sync.dma_start(
                            out_d[
                                qt * P : (qt + 1) * P,
                                hc * HCW : (hc + 1) * HCW,
                            ],
                            outst[:],
                        )

            aoT_free()
            wo_free()

    nc.compile()
    return nc


def _ensure_ntff_hook():
    """The container's antenv lacks axon_hooks; shim it and install the
    ctypes-based NTFF profile hook so trace=True works under axon."""
    try:
        from antenv.axon_hooks import get_axon_ntff_profile_hook  # noqa: F401

        return
    except ImportError:
        pass
    import sys
    import types

    mod = types.ModuleType("antenv.axon_hooks")
    mod._hook = None

    def set_axon_ntff_profile_hook(h):
        mod._hook = h

    def get_axon_ntff_profile_hook():
        return mod._hook

    mod.set_axon_ntff_profile_hook = set_axon_ntff_profile_hook
    mod.get_axon_ntff_profile_hook = get_axon_ntff_profile_hook
    sys.modules["antenv.axon_hooks"] = mod
    try:
        import antenv

        antenv.axon_hooks = mod
    except ImportError:
        pass
    try:
        from trn_agent_boot.trn_boot import _ntff_profile_via_ctypes

        set_axon_ntff_profile_hook(
            _ntff_profile_via_ctypes("/opt/axon/libaxon_pjrt.so")
        )
    except Exception:
        pass


_NC_CACHE = None


def _get_nc():
    global _NC_CACHE
    if _NC_CACHE is None:
        _NC_CACHE = build_nc()
    return _NC_CACHE


def kernel(hidden_states, cos, sin, w_qkv, w_o):
    hidden_states = np.asarray(hidden_states, dtype=np.float32)
    cos = np.asarray(cos, dtype=np.float32)
    sin = np.asarray(sin, dtype=np.float32)
    w_qkv = np.asarray(w_qkv, dtype=np.float32)
    w_o = np.asarray(w_o, dtype=np.float32)

    B = hidden_states.shape[0]
    assert hidden_states.shape == (B, S, HID)

    sin_pm = np.concatenate([-sin[:, : D // 2], sin[:, D // 2 :]], axis=1)
    sinTpm = np.ascontiguousarray(sin_pm.T.astype(ml_dtypes.bfloat16))
    cosT = np.ascontiguousarray(cos.T.astype(ml_dtypes.bfloat16))
    xT = [
        np.ascontiguousarray(hidden_states[b].T.astype(ml_dtypes.bfloat16))
        for b in range(B)
    ]
    wkv = w_qkv[:, H * D :]
    in_maps = []
    for b in range(B):
        for g in range(4):
            wcat = np.ascontiguousarray(
                np.concatenate(
                    [w_qkv[:, g * QCOLS : (g + 1) * QCOLS], wkv], axis=1
                ).astype(ml_dtypes.bfloat16)
            )
            wo_g = np.ascontiguousarray(
                w_o[g * QCOLS : (g + 1) * QCOLS, :].astype(
                    ml_dtypes.bfloat16
                )
            )
            in_maps.append(
                {
                    "xT": xT[b],
                    "wcat": wcat,
                    "wo": wo_g,
                    "cosT": cosT,
                    "sinTpm": sinTpm,
                }
            )

    nc = _get_nc()
    trace = bool(int(os.environ.get("EBT_TRACE", "0")))
    if trace:
        _ensure_ntff_hook()
    res = run_bass_kernel_spmd(
        nc, in_maps, core_ids=list(range(8)), trace=trace
    )
    if trace and res.exec_time_ns is not None:
        print(f"HW exec time: {res.exec_time_ns} ns")
        print(f"mean exec time: {res.mean_exec_time_ns} ns")
        if res.instructions_and_trace is not None:
            print(f"trace: {res.instructions_and_trace[1]}")

    parts = [np.asarray(r["out"], dtype=np.float32) for r in res.results]
    out = np.stack(
        [
            parts[0] + parts[1] + parts[2] + parts[3],
            parts[4] + parts[5] + parts[6] + parts[7],
        ],
        axis=0,
    )
    return out.astype(np.float32)

